# revision 6
# baseline (speedup 1.0000x reference)
"""Location-sensitive attention (Tacotron-style) on 8 TRN2 NeuronCores.

Data-parallel over batch: each core handles B=8 batch items, weights
replicated. Per core (b in [0,8)):
  pq[b,a]   = hidden[b] @ Wq.T                  (folded: W2pq row 62)
  loc2[t,a] = im2col(aw_cat)[62,t] @ W2[62,a]   (conv+Wp fused on host)
  e[b,t]    = sum_a Wv[a] * tanh(loc2 + pq + pm)
  aw        = softmax_t(e)  (no max-sub: |e| <= ||Wv||_1 ~ 10, exp safe)
  ctx[b,d]  = sum_t aw[t] * memory[t,d]

Global t-index mapping t = l*16 + j (l: partition 0..127, j: chunk 0..15)
so pm/memory load as single contiguous fat DMAs ([128, 16, A/D] tiles with
8KB/32KB descriptor runs), the conv matmul reads im2col columns via a
strided AP view, and softmax runs in column form (partition sums via
ones-matmul), leaving aw as ready-made matmul lhsT columns for the
context reduction.
"""

import numpy as np

import concourse.bass as bass
import concourse.mybir as mybir
from concourse import masks, tile
from concourse.ap import AP
from concourse.bass_utils import run_bass_kernel_spmd

N_CORES = 8
B_FULL, T = 64, 2048
B = B_FULL // N_CORES          # 8 batch items per core
RNN, EMB, ATT = 1024, 512, 128
NF, KS = 32, 31
PAD = (KS - 1) // 2            # 15
KC = 2 * KS                    # 62 im2col rows (c, dk)
NJ = 16                        # chunks; t = l*NJ + j
NL = T // NJ                   # 128 partitions
PADW = T + 2 * PAD             # 2078
F32 = mybir.dt.float32
AF = mybir.ActivationFunctionType
ALU = mybir.AluOpType

# set by test harness; graded path keeps defaults
TRACE = False
TRACE_DIR = None
LAST_RESULT = None


def split_sync_waits(nc: bass.Bass, cap: int = 1) -> bass.Bass:
    """Hoist attached multi-waits into standalone InstEventSemaphore ops.

    This walrus build accepts at most one attached sync-wait per
    instruction ("Too many sync wait commands" otherwise); Tile's
    add_semaphores freely attaches several. Standalone event-semaphore
    waits on the same engine are semantically identical and compile.
    """
    for f in nc.m.functions:
        for blk in f.blocks:
            old = list(blk.instructions)
            new = []
            for inst in old:
                si = inst.sync_info
                waits = list(si.on_wait) if si is not None and si.on_wait else []
                if len(waits) > cap:
                    extra, keep = waits[:-cap], waits[-cap:]
                    for k, w in enumerate(extra):
                        ev = mybir.InstEventSemaphore(
                            name=f"{inst.name}-w{k}", ins=[], outs=[]
                        )
                        ev.engine = inst.engine
                        ev.sync_info = mybir.SyncInfo(on_wait=[w], on_update=[])
                        new.append(ev)
                    inst.sync_info = mybir.SyncInfo(
                        on_wait=keep, on_update=list(si.on_update or [])
                    )
                new.append(inst)
            blk.instructions[:] = new
    return nc


def build_nc(split: bool = True) -> bass.Bass:
    nc = bass.Bass()
    hid = nc.declare_dram_parameter("hid", [B, RNN], F32, isOutput=False)
    mem = nc.declare_dram_parameter("mem", [B, T, EMB], F32, isOutput=False)
    pm = nc.declare_dram_parameter("pm", [B, T, ATT], F32, isOutput=False)
    awp = nc.declare_dram_parameter("awp", [B, 2, PADW], F32, isOutput=False)
    wqt = nc.declare_dram_parameter("wqt", [RNN, ATT], F32, isOutput=False)
    w2 = nc.declare_dram_parameter("w2", [KC, ATT], F32, isOutput=False)
    wv = nc.declare_dram_parameter("wv", [1, ATT], F32, isOutput=False)
    ctxo = nc.declare_dram_parameter("ctx_out", [B, EMB], F32, isOutput=True)
    awo = nc.declare_dram_parameter("aw_out", [B, T], F32, isOutput=True)

    with tile.TileContext(nc) as tc:
        with (
            tc.tile_pool(name="const", bufs=1) as cpool,
            tc.tile_pool(name="stream", bufs=2) as spool,
            tc.tile_pool(name="work", bufs=3) as wpool,
            tc.tile_pool(name="ps_loc", bufs=2, space=bass.MemorySpace.PSUM) as ps_loc,
            tc.tile_pool(name="ps_ctx", bufs=2, space=bass.MemorySpace.PSUM) as ps_ctx,
            tc.tile_pool(name="ps_sm", bufs=2, space=bass.MemorySpace.PSUM) as ps_sm,
        ):
            # ---------- constants / prologue ----------
            ident = cpool.tile([128, 128], F32)
            masks.make_identity(nc, ident[:])
            ones_col = cpool.tile([128, 1], F32)
            nc.vector.memset(ones_col[:], 1.0)
            ones_row = cpool.tile([1, 128], F32)
            nc.vector.memset(ones_row[:], 1.0)
            wv_bc = cpool.tile([128, ATT], F32)
            nc.sync.dma_start(out=wv_bc[:], in_=AP(wv, 0, [[0, 128], [1, ATT]]))

            wqt_sb = cpool.tile([128, RNN // 128, ATT], F32)
            nc.sync.dma_start(
                out=wqt_sb[:], in_=wqt[:].rearrange("(kc p) a -> p kc a", p=128)
            )
            hid_sb = cpool.tile([B, RNN], F32)
            nc.sync.dma_start(out=hid_sb[:], in_=hid[:])

            # hidden^T (128k x 8b chunks) via PE transpose
            hT = cpool.tile([128, RNN // 128, B], F32)
            for kc in range(RNN // 128):
                pst = ps_loc.tile([128, B], F32, tag="loc")
                nc.tensor.transpose(
                    pst[:], hid_sb[:, kc * 128 : (kc + 1) * 128], ident[:B, :B]
                )
                nc.scalar.copy(out=hT[:, kc, :], in_=pst[:])

            # pq[b, a] = sum_r hid[b, r] wqt[r, a]
            ps_pq = ps_ctx.tile([B, ATT], F32, tag="ctx")
            for kc in range(RNN // 128):
                nc.tensor.matmul(
                    ps_pq[:],
                    hT[:, kc, :],
                    wqt_sb[:, kc, :],
                    start=(kc == 0),
                    stop=(kc == RNN // 128 - 1),
                )
            pq_sb = cpool.tile([B, ATT], F32)
            nc.vector.tensor_copy(pq_sb[:], ps_pq[:])

            # per-b conv weights [63, A]: row 0 = pq[b], rows 1..62 = W2
            w2pq = []
            for b in range(B):
                t_ = cpool.tile([KC + 1, ATT], F32, tag=f"w2pq{b}")
                nc.sync.dma_start(out=t_[1 : KC + 1, :], in_=w2[:])
                # partition shift b -> 0 must go through DMA
                nc.sync.dma_start(out=t_[0:1, :], in_=pq_sb[b : b + 1, :])
                w2pq.append(t_)

            aw_view = awo[:].rearrange("b (l j) -> b l j", j=NJ)

            # ---------- main loop over batch ----------
            for b in range(B):
                # im2col: xp[c*31+dk, t] = awp[b, c, t+dk]  (pre-shifted rows)
                xp = wpool.tile([KC + 1, T], F32, tag="xp")
                nc.sync.dma_start(
                    out=xp[1 : KC + 1, :],
                    in_=AP(awp, b * 2 * PADW, [[PADW, 2], [1, KS], [1, T]]),
                )
                nc.vector.memset(xp[0:1, :], 1.0)  # ones row -> pq term

                pmt = spool.tile([NL, NJ, ATT], F32, tag="pm")
                nc.sync.dma_start(
                    out=pmt[:], in_=pm[b].rearrange("(l j) a -> l j a", j=NJ)
                )
                memt = spool.tile([NL, NJ, EMB], F32, tag="mem")
                nc.sync.dma_start(
                    out=memt[:], in_=mem[b].rearrange("(l j) d -> l j d", j=NJ)
                )

                xp3 = xp[:].rearrange("k (l j) -> k l j", j=NJ)  # [63, 128, 16]
                ecol = wpool.tile([NL, NJ], F32, tag="ecol")

                QG = 4  # chunks fused per DVE/ACT op
                for jg in range(NJ // QG):
                    psl = ps_loc.tile([NL, QG * ATT], F32, tag="loc")
                    for q in range(QG):
                        j = jg * QG + q
                        nc.tensor.matmul(
                            psl[:, q * ATT : (q + 1) * ATT],
                            xp3[:, :, j],
                            w2pq[b][:],
                            start=True,
                            stop=True,
                        )
                    tharg = wpool.tile([NL, QG * ATT], F32, tag="tharg")
                    nc.vector.tensor_add(
                        tharg[:],
                        psl[:],
                        pmt[:, jg * QG : (jg + 1) * QG, :].rearrange(
                            "l q a -> l (q a)"
                        ),
                    )
                    tho = wpool.tile([NL, QG * ATT], F32, tag="tho")
                    nc.scalar.activation(tho[:], tharg[:], AF.Tanh)
                    for q in range(QG):
                        j = jg * QG + q
                        nc.vector.scalar_tensor_tensor(
                            out=tharg[:, q * ATT : (q + 1) * ATT],
                            in0=tho[:, q * ATT : (q + 1) * ATT],
                            scalar=1.0,
                            in1=wv_bc[:],
                            op0=ALU.mult,
                            op1=ALU.mult,
                            accum_out=ecol[:, j : j + 1],
                        )

                # softmax over t, column form [128l, 16j]
                exi = wpool.tile([NL, NJ], F32, tag="exi")
                nc.scalar.activation(exi[:], ecol[:], AF.Exp)
                pss = ps_sm.tile([1, NJ], F32, tag="sm")
                nc.tensor.matmul(pss[:], ones_col[:], exi[:], start=True, stop=True)
                ssum = wpool.tile([1, 1], F32, tag="ssum")
                nc.vector.reduce_sum(ssum[:], pss[:], axis=mybir.AxisListType.X)
                rin = wpool.tile([1, 1], F32, tag="rin")
                nc.vector.reciprocal(rin[:], ssum[:])
                psr = ps_sm.tile([128, 1], F32, tag="smr")
                nc.tensor.matmul(psr[:], ones_row[:], rin[:], start=True, stop=True)
                rcol = wpool.tile([128, 1], F32, tag="rcol")
                nc.vector.tensor_copy(rcol[:], psr[:])
                awcb = wpool.tile([NL, NJ], F32, tag="awc")
                nc.vector.tensor_scalar_mul(awcb[:], exi[:], rcol[:])

                nc.sync.dma_start(out=aw_view[b], in_=awcb[:])

                # context: ctx[d] = sum_t aw[t] mem[t, d], K-chunked over j
                psc = ps_ctx.tile([1, EMB], F32, tag="ctx")
                for j in range(NJ):
                    nc.tensor.matmul(
                        psc[:],
                        awcb[:, j : j + 1],
                        memt[:, j, :],
                        start=(j == 0),
                        stop=(j == NJ - 1),
                    )
                ctx_row = wpool.tile([1, EMB], F32, tag="ctxrow")
                nc.scalar.copy(out=ctx_row[:], in_=psc[:])
                nc.sync.dma_start(out=ctxo[b], in_=ctx_row[:])

    if split:
        split_sync_waits(nc)
    return nc


_cached_nc = None


def _get_nc():
    global _cached_nc
    if _cached_nc is None:
        _cached_nc = build_nc()
    return _cached_nc


def prep_inputs(attention_hidden_state, memory, processed_memory,
                attention_weights_cat, Wq, conv_w, Wp, Wv):
    hs = np.ascontiguousarray(np.asarray(attention_hidden_state, np.float32))
    mem = np.ascontiguousarray(np.asarray(memory, np.float32))
    pm = np.ascontiguousarray(np.asarray(processed_memory, np.float32))
    awc = np.asarray(attention_weights_cat, np.float32)
    wq = np.asarray(Wq, np.float32)
    cw = np.asarray(conv_w, np.float32)
    wp = np.asarray(Wp, np.float32)
    wvv = np.asarray(Wv, np.float32)

    wqt = np.ascontiguousarray(wq.T)                       # (1024, 128)
    # W2[(c,dk), a] = sum_f Wp[a,f] conv_w[f,c,dk]
    w2 = np.ascontiguousarray(
        np.einsum("af,fck->cka", wp, cw).reshape(KC, ATT)
    )
    wvr = np.ascontiguousarray(wvv.reshape(1, ATT))
    awp = np.zeros((B_FULL, 2, PADW), np.float32)
    awp[:, :, PAD : PAD + T] = awc

    in_maps = []
    for i in range(N_CORES):
        sl = slice(i * B, (i + 1) * B)
        in_maps.append(
            {
                "hid": hs[sl],
                "mem": mem[sl],
                "pm": pm[sl],
                "awp": np.ascontiguousarray(awp[sl]),
                "wqt": wqt,
                "w2": w2,
                "wv": wvr,
            }
        )
    return in_maps


def kernel(attention_hidden_state, memory, processed_memory,
           attention_weights_cat, mask, Wq, conv_w, Wp, Wv):
    global LAST_RESULT
    in_maps = prep_inputs(attention_hidden_state, memory, processed_memory,
                          attention_weights_cat, Wq, conv_w, Wp, Wv)
    nc = _get_nc()
    res = run_bass_kernel_spmd(
        nc,
        in_maps,
        list(range(N_CORES)),
        trace=TRACE,
        tmpdir=TRACE_DIR,
    )
    LAST_RESULT = res
    ctx = np.concatenate([res.results[i]["ctx_out"] for i in range(N_CORES)], 0)
    aw = np.concatenate([res.results[i]["aw_out"] for i in range(N_CORES)], 0)
    return ctx, aw


# revision 8
# speedup vs baseline: 1.2057x; 1.2057x over previous
"""Location-sensitive attention (Tacotron-style) on 8 TRN2 NeuronCores.

Data-parallel over batch: each core handles B=8 batch items, weights
replicated. Per core (b in [0,8)):
  pq[b,a]   = hidden[b] @ Wq.T                  (folded: W2pq row 0)
  loc2[t,a] = im2col(aw_cat)[62,t] @ W2[62,a]   (conv+Wp fused on host)
  e[b,t]    = sum_a Wv[a] * tanh(loc2 + pq + pm)
  aw        = softmax_t(e)  (no max-sub: |e| <= ||Wv||_1 ~ 10, exp safe)
  ctx[b,d]  = sum_t aw[t] * memory[t,d]

Global t-index mapping t = l*16 + j (l: partition 0..127, j: chunk 0..15)
so pm/memory load as single contiguous fat DMAs, the conv matmul reads
im2col columns via a strided AP view, and softmax runs in column form
(partition sums via ones-matmul), leaving aw as ready-made matmul lhsT
columns for the context reduction.

The conv path (im2col + W2 + pq row) runs in bf16: the conv operands are
attention weights in [0,1] and tiny learned filters, so bf16 rounding
perturbs energies by ~1e-3 relative, far under the accuracy gate, while
halving both the im2col DMA traffic and the loc-matmul PE stream time.

DMA issue engines are spread (sync=memory, scalar=pm, gpsimd=im2col,
vector=small I/O) so descriptors land on different queue rows instead of
piling onto SDMA engines 0/1 via the single SP HWDGE ring.
"""

import numpy as np
import ml_dtypes

import concourse.bass as bass
import concourse.mybir as mybir
from concourse import masks, tile
from concourse.ap import AP
from concourse.bass_utils import run_bass_kernel_spmd

N_CORES = 8
B_FULL, T = 64, 2048
B = B_FULL // N_CORES          # 8 batch items per core
RNN, EMB, ATT = 1024, 512, 128
NF, KS = 32, 31
PAD = (KS - 1) // 2            # 15
KC = 2 * KS                    # 62 im2col rows (c, dk)
NJ = 16                        # chunks; t = l*NJ + j
NL = T // NJ                   # 128 partitions
PADW = T + 2 * PAD             # 2078
F32 = mybir.dt.float32
BF16 = mybir.dt.bfloat16
AF = mybir.ActivationFunctionType
ALU = mybir.AluOpType

# set by test harness; graded path keeps defaults
TRACE = False
TRACE_DIR = None
LAST_RESULT = None


def split_sync_waits(nc: bass.Bass, cap: int = 1) -> bass.Bass:
    """Hoist attached multi-waits into standalone InstEventSemaphore ops.

    This walrus build accepts at most one attached sync-wait per
    instruction ("Too many sync wait commands" otherwise); Tile's
    add_semaphores freely attaches several. Standalone event-semaphore
    waits on the same engine are semantically identical and compile.
    """
    for f in nc.m.functions:
        for blk in f.blocks:
            old = list(blk.instructions)
            new = []
            for inst in old:
                si = inst.sync_info
                waits = list(si.on_wait) if si is not None and si.on_wait else []
                if len(waits) > cap:
                    extra, keep = waits[:-cap], waits[-cap:]
                    for k, w in enumerate(extra):
                        ev = mybir.InstEventSemaphore(
                            name=f"{inst.name}-w{k}", ins=[], outs=[]
                        )
                        ev.engine = inst.engine
                        ev.sync_info = mybir.SyncInfo(on_wait=[w], on_update=[])
                        new.append(ev)
                    inst.sync_info = mybir.SyncInfo(
                        on_wait=keep, on_update=list(si.on_update or [])
                    )
                new.append(inst)
            blk.instructions[:] = new
    return nc


def build_nc(split: bool = True) -> bass.Bass:
    nc = bass.Bass()
    hid = nc.declare_dram_parameter("hid", [B, RNN], F32, isOutput=False)
    mem = nc.declare_dram_parameter("mem", [B, T, EMB], F32, isOutput=False)
    pm = nc.declare_dram_parameter("pm", [B, T, ATT], F32, isOutput=False)
    awp = nc.declare_dram_parameter("awp", [B, 2, PADW], BF16, isOutput=False)
    wqt = nc.declare_dram_parameter("wqt", [RNN, ATT], F32, isOutput=False)
    w2 = nc.declare_dram_parameter("w2", [KC, ATT], BF16, isOutput=False)
    wv = nc.declare_dram_parameter("wv", [1, ATT], F32, isOutput=False)
    ctxo = nc.declare_dram_parameter("ctx_out", [B, EMB], F32, isOutput=True)
    awo = nc.declare_dram_parameter("aw_out", [B, T], F32, isOutput=True)

    with tile.TileContext(nc) as tc:
        with (
            tc.tile_pool(name="const", bufs=1) as cpool,
            tc.tile_pool(name="stream", bufs=3) as spool,
            tc.tile_pool(name="work", bufs=3) as wpool,
            tc.tile_pool(name="ps_loc", bufs=3, space=bass.MemorySpace.PSUM) as ps_loc,
            tc.tile_pool(name="ps_ctx", bufs=2, space=bass.MemorySpace.PSUM) as ps_ctx,
            tc.tile_pool(name="ps_sm", bufs=2, space=bass.MemorySpace.PSUM) as ps_sm,
        ):
            # ---------- constants / prologue ----------
            ident = cpool.tile([128, 128], F32)
            masks.make_identity(nc, ident[:])
            ones_col = cpool.tile([128, 1], F32)
            nc.gpsimd.memset(ones_col[:], 1.0)
            ones_row = cpool.tile([1, 128], F32)
            nc.gpsimd.memset(ones_row[:], 1.0)
            wv_bc = cpool.tile([128, ATT], F32)
            nc.gpsimd.dma_start(out=wv_bc[:], in_=AP(wv, 0, [[0, 128], [1, ATT]]))

            wqt_sb = cpool.tile([128, RNN // 128, ATT], F32)
            nc.gpsimd.dma_start(
                out=wqt_sb[:], in_=wqt[:].rearrange("(kc p) a -> p kc a", p=128)
            )
            hid_sb = cpool.tile([B, RNN], F32)
            nc.gpsimd.dma_start(out=hid_sb[:], in_=hid[:])

            # hidden^T (128k x 8b chunks) via PE transpose
            hT = cpool.tile([128, RNN // 128, B], F32)
            for kc in range(RNN // 128):
                pst = ps_loc.tile([128, B], F32, tag="loc")
                nc.tensor.transpose(
                    pst[:], hid_sb[:, kc * 128 : (kc + 1) * 128], ident[:B, :B]
                )
                nc.scalar.copy(out=hT[:, kc, :], in_=pst[:])

            # pq[b, a] = sum_r hid[b, r] wqt[r, a]
            ps_pq = ps_ctx.tile([B, ATT], F32, tag="ctx")
            for kc in range(RNN // 128):
                nc.tensor.matmul(
                    ps_pq[:],
                    hT[:, kc, :],
                    wqt_sb[:, kc, :],
                    start=(kc == 0),
                    stop=(kc == RNN // 128 - 1),
                )
            pq_sb = cpool.tile([B, ATT], BF16)
            nc.vector.tensor_copy(pq_sb[:], ps_pq[:])  # f32 -> bf16 cast

            # per-b conv weights [63, A] bf16: row 0 = pq[b], rows 1..62 = W2
            w2pq = []
            for b in range(B):
                t_ = cpool.tile([KC + 1, ATT], BF16, tag=f"w2pq{b}")
                nc.gpsimd.dma_start(out=t_[1 : KC + 1, :], in_=w2[:])
                # partition shift b -> 0 must go through DMA
                nc.gpsimd.dma_start(out=t_[0:1, :], in_=pq_sb[b : b + 1, :])
                w2pq.append(t_)

            aw_view = awo[:].rearrange("b (l j) -> b l j", j=NJ)

            # ---------- main loop over batch ----------
            for b in range(B):
                # im2col: xp[1 + c*31+dk, t] = awp[b, c, t+dk]  (pre-shifted rows)
                xp = wpool.tile([KC + 1, T], BF16, tag="xp")
                nc.gpsimd.dma_start(
                    out=xp[1 : KC + 1, :],
                    in_=AP(awp, b * 2 * PADW, [[PADW, 2], [1, KS], [1, T]]),
                )
                nc.gpsimd.memset(xp[0:1, :], 1.0)  # ones row -> pq term

                pmt = spool.tile([NL, NJ, ATT], F32, tag="pm")
                nc.scalar.dma_start(
                    out=pmt[:], in_=pm[b].rearrange("(l j) a -> l j a", j=NJ)
                )
                memt = spool.tile([NL, NJ, EMB], F32, tag="mem")
                nc.sync.dma_start(
                    out=memt[:], in_=mem[b].rearrange("(l j) d -> l j d", j=NJ)
                )

                xp3 = xp[:].rearrange("k (l j) -> k l j", j=NJ)  # [63, 128, 16]
                ecol = wpool.tile([NL, NJ], F32, tag="ecol")

                QG = 4  # chunks fused per DVE/ACT op
                for jg in range(NJ // QG):
                    psl = ps_loc.tile([NL, QG * ATT], F32, tag="loc")
                    for q in range(QG):
                        j = jg * QG + q
                        nc.tensor.matmul(
                            psl[:, q * ATT : (q + 1) * ATT],
                            xp3[:, :, j],
                            w2pq[b][:],
                            start=True,
                            stop=True,
                        )
                    tharg = wpool.tile([NL, QG * ATT], F32, tag="tharg")
                    nc.vector.tensor_add(
                        tharg[:],
                        psl[:],
                        pmt[:, jg * QG : (jg + 1) * QG, :].rearrange(
                            "l q a -> l (q a)"
                        ),
                    )
                    tho = wpool.tile([NL, QG * ATT], F32, tag="tho")
                    nc.scalar.activation(tho[:], tharg[:], AF.Tanh)
                    for q in range(QG):
                        j = jg * QG + q
                        nc.vector.scalar_tensor_tensor(
                            out=tharg[:, q * ATT : (q + 1) * ATT],
                            in0=tho[:, q * ATT : (q + 1) * ATT],
                            scalar=1.0,
                            in1=wv_bc[:],
                            op0=ALU.mult,
                            op1=ALU.mult,
                            accum_out=ecol[:, j : j + 1],
                        )

                # softmax over t, column form [128l, 16j]
                exi = wpool.tile([NL, NJ], F32, tag="exi")
                nc.scalar.activation(exi[:], ecol[:], AF.Exp)
                pss = ps_sm.tile([1, NJ], F32, tag="sm")
                nc.tensor.matmul(pss[:], ones_col[:], exi[:], start=True, stop=True)
                ssum = wpool.tile([1, 1], F32, tag="ssum")
                nc.vector.reduce_sum(ssum[:], pss[:], axis=mybir.AxisListType.X)
                rin = wpool.tile([1, 1], F32, tag="rin")
                nc.vector.reciprocal(rin[:], ssum[:])
                psr = ps_sm.tile([128, 1], F32, tag="sm")
                nc.tensor.matmul(psr[:], ones_row[:], rin[:], start=True, stop=True)
                rcol = wpool.tile([128, 1], F32, tag="rcol")
                nc.vector.tensor_copy(rcol[:], psr[:])
                awcb = wpool.tile([NL, NJ], F32, tag="awc")
                nc.vector.tensor_scalar_mul(awcb[:], exi[:], rcol[:])

                nc.scalar.dma_start(out=aw_view[b], in_=awcb[:])

                # context: ctx[d] = sum_t aw[t] mem[t, d], K-chunked over j
                psc = ps_ctx.tile([1, EMB], F32, tag="ctx")
                for j in range(NJ):
                    nc.tensor.matmul(
                        psc[:],
                        awcb[:, j : j + 1],
                        memt[:, j, :],
                        start=(j == 0),
                        stop=(j == NJ - 1),
                    )
                ctx_row = wpool.tile([1, EMB], F32, tag="ctxrow")
                nc.scalar.copy(out=ctx_row[:], in_=psc[:])
                nc.scalar.dma_start(out=ctxo[b], in_=ctx_row[:])

    if split:
        split_sync_waits(nc)
    return nc


_cached_nc = None


def _get_nc():
    global _cached_nc
    if _cached_nc is None:
        _cached_nc = build_nc()
    return _cached_nc


def prep_inputs(attention_hidden_state, memory, processed_memory,
                attention_weights_cat, Wq, conv_w, Wp, Wv):
    hs = np.ascontiguousarray(np.asarray(attention_hidden_state, np.float32))
    mem = np.ascontiguousarray(np.asarray(memory, np.float32))
    pm = np.ascontiguousarray(np.asarray(processed_memory, np.float32))
    awc = np.asarray(attention_weights_cat, np.float32)
    wq = np.asarray(Wq, np.float32)
    cw = np.asarray(conv_w, np.float32)
    wp = np.asarray(Wp, np.float32)
    wvv = np.asarray(Wv, np.float32)

    wqt = np.ascontiguousarray(wq.T)                       # (1024, 128)
    # W2[(c,dk), a] = sum_f Wp[a,f] conv_w[f,c,dk]
    w2 = np.ascontiguousarray(
        np.einsum("af,fck->cka", wp, cw).reshape(KC, ATT)
    ).astype(ml_dtypes.bfloat16)
    wvr = np.ascontiguousarray(wvv.reshape(1, ATT))
    awp = np.zeros((B_FULL, 2, PADW), np.float32)
    awp[:, :, PAD : PAD + T] = awc
    awp = awp.astype(ml_dtypes.bfloat16)

    in_maps = []
    for i in range(N_CORES):
        sl = slice(i * B, (i + 1) * B)
        in_maps.append(
            {
                "hid": hs[sl],
                "mem": mem[sl],
                "pm": pm[sl],
                "awp": np.ascontiguousarray(awp[sl]),
                "wqt": wqt,
                "w2": w2,
                "wv": wvr,
            }
        )
    return in_maps


def kernel(attention_hidden_state, memory, processed_memory,
           attention_weights_cat, mask, Wq, conv_w, Wp, Wv):
    global LAST_RESULT
    in_maps = prep_inputs(attention_hidden_state, memory, processed_memory,
                          attention_weights_cat, Wq, conv_w, Wp, Wv)
    nc = _get_nc()
    res = run_bass_kernel_spmd(
        nc,
        in_maps,
        list(range(N_CORES)),
        trace=TRACE,
        tmpdir=TRACE_DIR,
    )
    LAST_RESULT = res
    ctx = np.concatenate([res.results[i]["ctx_out"] for i in range(N_CORES)], 0)
    aw = np.concatenate([res.results[i]["aw_out"] for i in range(N_CORES)], 0)
    return ctx, aw
